# revision 1
# baseline (speedup 1.0000x reference)
"""Trainium2 Bass kernel for nn_CustomCrossModalAttention (B=2, N=2048, D=768, H=12).

Sharding (8 cores, zero redundant matmul work):
  - core c owns batch b = c//4 and query rows [512*(c%4), 512*(c%4)+512) of that batch.
  - Phase 1 (row-parallel): each core computes q, k, v projections + LayerNorm for its
    512 rows only. k is folded with the positional term: the reference computes
    scores = (q@k^T)*scale + q@pos^T == scale * (q @ (k + pos/scale)^T), so we build
    k' = LN_k(xk) + pos/scale once.
  - Two AllGathers per 4-core batch group exchange the k'^T and v shards (k first so
    score matmuls can start while v is still in flight).
  - Phase 2 (row-parallel): 12-head attention on the core's 512 query rows with
    softmax (exp without max-subtraction; row sums via a ones column appended to v),
    then output proj, gate, fuse and final LayerNorm.

Precision: projections and the output projection run in float32r (TF32-like, same
PE throughput as bf16 at moving-dim >= 256); attention internals and the gate run
in bf16; all accumulation fp32.

Algebraic folds done on the host (all exact):
  - LN(v) gain/bias folded into wo / bo (uses sum_m attn[n,m] == 1 post-normalize).
  - q/k LN gain+bias applied during the PE-transpose copy-out (per-partition scalars
    in the transposed layout).
  - All matmul biases applied as an extra K=1 matmul with a ones row.
"""

import numpy as np
import ml_dtypes

B, N, D = 2, 2048, 768
H, DH = 12, 64
P = 128
CORES, GROUP = 8, 4
S = 512            # query rows per core
NCH = S // P       # 4 row chunks per core
MCH = N // P       # 16 key chunks
G6 = D // P        # 6
SCALE = DH ** -0.5
EPS = 1e-5

BF = ml_dtypes.bfloat16

_CACHE = {}


def _build():
    from contextlib import ExitStack

    import concourse.bacc as bacc
    import concourse.mybir as mybir
    import concourse.tile as tile
    from concourse.masks import make_identity

    f32 = mybir.dt.float32
    f32r = mybir.dt.float32r
    bf16 = mybir.dt.bfloat16
    ALU = mybir.AluOpType
    ACTF = mybir.ActivationFunctionType

    nc = bacc.Bacc("TRN2", target_bir_lowering=False, num_devices=CORES)

    def din(name, shape, dt=bf16):
        return nc.dram_tensor(name, shape, dt, kind="ExternalInput")

    xqT = din("xqT", [D, S], f32r)      # infrared rows, transposed
    xvT = din("xvT", [D, S], f32r)      # visible rows, transposed
    vis_nat = din("vis_nat", [S, D], f32)
    posTb = din("posTb", [D, S])        # pos/scale + lnk_b, transposed (bf16)
    wqkvT = din("wqkvT", [D, 3 * D], f32r)
    bqkv = din("bqkv", [1, 3 * D], f32r)
    woT = din("woT", [D, D], f32r)      # (wo * lnv_w).T
    bo_a = din("bo_a", [1, D], f32r)    # bo + wo @ lnv_b
    gwT = din("gwT", [2 * D, D], f32r)
    gb = din("gb", [1, D], f32r)
    lnq_g = din("lnq_g", [P, G6], f32)
    lnq_b = din("lnq_b", [P, G6], f32)
    lnk_g = din("lnk_g", [P, G6], f32)
    lnf = din("lnf", [2, D], f32)
    out_rows = nc.dram_tensor("out_rows", [S, D], f32, kind="ExternalOutput")

    FLK = D * S                      # k'^T payload
    FLV = NCH * P * H * (DH + 1)     # v payload (padded with ones col)
    cc_in_k = nc.dram_tensor("cc_in_k", [FLK], f32r)
    cc_out_k = nc.dram_tensor("cc_out_k", [GROUP, FLK], f32r)
    cc_in_v = nc.dram_tensor("cc_in_v", [FLV], bf16)
    cc_out_v = nc.dram_tensor("cc_out_v", [GROUP, FLV], bf16)
    groups = [[0, 1, 2, 3], [4, 5, 6, 7]]

    HALves = [(0, 512), (512, D)]

    with tile.TileContext(nc) as tc, ExitStack() as ctx:
        const = ctx.enter_context(tc.tile_pool(name="const", bufs=1))
        persist = ctx.enter_context(tc.tile_pool(name="persist", bufs=1))

        ident = const.tile([P, P], bf16)
        make_identity(nc, ident)
        ident_f32 = const.tile([P, P], f32)
        make_identity(nc, ident_f32)
        ones_r_f = const.tile([1, P], f32)
        nc.vector.memset(ones_r_f, 1.0)
        ones_r = ones_r_f.bitcast(f32r)
        ones_bf = const.tile([1, P], bf16)
        nc.vector.memset(ones_bf, 1.0)
        ones_f32 = const.tile([1, P], f32)
        nc.vector.memset(ones_f32, 1.0)
        eps_t = const.tile([P, 1], f32)
        nc.vector.memset(eps_t, EPS)

        xvT_sb = const.tile([P, G6, S], f32r)
        nc.gpsimd.dma_start(out=xvT_sb, in_=xvT.rearrange("(s p) n -> p s n", p=P))
        woT_sb = const.tile([P, G6, D], f32r)
        nc.scalar.dma_start(out=woT_sb, in_=woT.rearrange("(s p) o -> p s o", p=P))
        bo_sb = const.tile([1, D], f32r)
        nc.sync.dma_start(out=bo_sb, in_=bo_a.ap())
        gb_sb = const.tile([1, D], f32r)
        nc.sync.dma_start(out=gb_sb, in_=gb.ap())
        lnq_g_sb = const.tile([P, G6], f32)
        nc.sync.dma_start(out=lnq_g_sb, in_=lnq_g.ap())
        lnq_b_sb = const.tile([P, G6], f32)
        nc.sync.dma_start(out=lnq_b_sb, in_=lnq_b.ap())
        lnk_g_sb = const.tile([P, G6], f32)
        nc.sync.dma_start(out=lnk_g_sb, in_=lnk_g.ap())
        lnfw_sb = const.tile([1, D], f32)
        nc.sync.dma_start(out=lnfw_sb, in_=lnf.ap()[0:1, :])
        lnfb_sb = const.tile([1, D], f32)
        nc.sync.dma_start(out=lnfb_sb, in_=lnf.ap()[1:2, :])

        outT_sb = persist.tile([P, G6, S], f32r)

        with tc.tile_pool(name="mid", bufs=1) as midp:
            qT_sb = midp.tile([P, G6, S], f32r)

            with (
                tc.tile_pool(name="ph1", bufs=1) as ph1,
                tc.tile_pool(name="wrot", bufs=2) as wrot,
                tc.tile_pool(name="pwork", bufs=1) as pwork,
                tc.tile_pool(name="stat", bufs=6) as stat,
                tc.tile_pool(name="psum_p", bufs=2, space="PSUM") as psum_p,
                tc.tile_pool(name="psum_t", bufs=2, space="PSUM") as psum_t,
            ):
                xqT_sb = ph1.tile([P, G6, S], f32r)
                nc.gpsimd.dma_start(
                    out=xqT_sb, in_=xqT.rearrange("(s p) n -> p s n", p=P)
                )
                bqkv_sb = ph1.tile([1, 3 * D], f32r)
                nc.sync.dma_start(out=bqkv_sb, in_=bqkv.ap())
                posTb_sb = ph1.tile([P, G6, S], bf16)
                nc.sync.dma_start(
                    out=posTb_sb, in_=posTb.rearrange("(s p) n -> p s n", p=P)
                )

                kloc_sb = ph1.tile([P, G6, S], f32r)   # local k'^T shard
                vloc_sb = ph1.tile([P, NCH, H, DH + 1], bf16)
                nc.vector.memset(vloc_sb[:, :, :, DH:DH + 1], 1.0)

                def load_w(off):
                    w_sb = wrot.tile([P, G6, D], f32r, tag="w")
                    nc.sync.dma_start(
                        out=w_sb,
                        in_=wqkvT.rearrange("(s p) o -> p s o", p=P)[:, :, off:off + D],
                    )
                    return w_sb

                def proj_tile(lhsT_sb, w_sb, w_off, c):
                    py = psum_p.tile([P, D], f32)
                    for o0, o1 in HALves:
                        for s in range(G6):
                            nc.tensor.matmul(
                                py[:, o0:o1],
                                lhsT_sb[:, s, c * P:(c + 1) * P],
                                w_sb[:, s, o0:o1],
                                start=(s == 0), stop=False,
                            )
                        nc.tensor.matmul(
                            py[:, o0:o1], ones_r,
                            bqkv_sb[:, w_off + o0:w_off + o1],
                            start=False, stop=True,
                        )
                    return py

                def ln_stats(y, pool):
                    st = pool.tile([P, 2, 6], f32)
                    for i in range(2):
                        nc.vector.bn_stats(
                            out=st[:, i], in_=y[:, i * 384:(i + 1) * 384]
                        )
                    mv = pool.tile([P, 2], f32)
                    nc.vector.bn_aggr(out=mv, in_=st)
                    rstd = pool.tile([P, 1], f32)
                    nc.scalar.activation(
                        out=rstd, in_=mv[:, 1:2], func=ACTF.Sqrt,
                        bias=eps_t, scale=1.0,
                    )
                    nc.vector.reciprocal(out=rstd, in_=rstd)
                    # negmr = -mu*rstd: ACT applies (y-mu)*rstd as y*rstd+negmr
                    negmr = pool.tile([P, 1], f32)
                    nc.vector.tensor_scalar(
                        out=negmr, in0=mv[:, 0:1], scalar1=rstd, scalar2=-1.0,
                        op0=ALU.mult, op1=ALU.mult,
                    )
                    return negmr, rstd

                # ---- k' ----
                wk_sb = load_w(D)
                knats = []
                for c in range(NCH):
                    py = proj_tile(xvT_sb, wk_sb, D, c)
                    negmr, rstd = ln_stats(py, stat)
                    knat = pwork.tile([P, D], f32, tag=f"knat{c}")
                    nc.scalar.activation(
                        out=knat, in_=py, func=ACTF.Identity,
                        bias=negmr, scale=rstd,
                    )
                    knats.append(knat)
                for s in range(G6):
                    pt = psum_t.tile([P, NCH, P], f32)
                    for c in range(NCH):
                        nc.tensor.transpose(
                            pt[:, c], knats[c][:, s * P:(s + 1) * P], ident_f32
                        )
                    nc.vector.scalar_tensor_tensor(
                        out=kloc_sb[:, s, :],
                        in0=pt.rearrange("p c n -> p (c n)"),
                        scalar=lnk_g_sb[:, s:s + 1],
                        in1=posTb_sb[:, s, :],
                        op0=ALU.mult, op1=ALU.add,
                    )

                # ---- exchange k' (scores need it first) ----
                nc.sync.dma_start(
                    out=cc_in_k.ap().rearrange("(s p n) -> p s n", p=P, s=G6),
                    in_=kloc_sb,
                )
                nc.gpsimd.collective_compute(
                    "AllGather", ALU.bypass, replica_groups=groups,
                    ins=[cc_in_k.ap().opt()], outs=[cc_out_k.ap().opt()],
                )
                # ---- v ----
                wv_sb = load_w(2 * D)
                for c in range(NCH):
                    py = proj_tile(xvT_sb, wv_sb, 2 * D, c)
                    negmr, rstd = ln_stats(py, stat)
                    nc.scalar.activation(
                        out=vloc_sb[:, c, :, 0:DH],
                        in_=py.rearrange("p (h d) -> p h d", h=H),
                        func=ACTF.Identity, bias=negmr, scale=rstd,
                    )

                nc.sync.dma_start(
                    out=cc_in_v.ap().rearrange("(c p f) -> p c f", c=NCH, p=P),
                    in_=vloc_sb.rearrange("p c h d -> p c (h d)"),
                )
                nc.gpsimd.collective_compute(
                    "AllGather", ALU.bypass, replica_groups=groups,
                    ins=[cc_in_v.ap().opt()], outs=[cc_out_v.ap().opt()],
                )
                # ---- q ----
                wq_sb = load_w(0)
                qnats = []
                for c in range(NCH):
                    py = proj_tile(xqT_sb, wq_sb, 0, c)
                    negmr, rstd = ln_stats(py, stat)
                    qnat = pwork.tile([P, D], f32, tag=f"qnat{c}")
                    nc.scalar.activation(
                        out=qnat, in_=py, func=ACTF.Identity,
                        bias=negmr, scale=rstd,
                    )
                    qnats.append(qnat)
                for s in range(G6):
                    pt = psum_t.tile([P, NCH, P], f32)
                    for c in range(NCH):
                        nc.tensor.transpose(
                            pt[:, c], qnats[c][:, s * P:(s + 1) * P], ident_f32
                        )
                    nc.vector.scalar_tensor_tensor(
                        out=qT_sb[:, s, :],
                        in0=pt.rearrange("p c n -> p (c n)"),
                        scalar=lnq_g_sb[:, s:s + 1],
                        in1=lnq_b_sb[:, s:s + 1].to_broadcast([P, S]),
                        op0=ALU.mult, op1=ALU.add,
                    )

            # ---- attention ----
            with (
                tc.tile_pool(name="gath", bufs=1) as gath,
                tc.tile_pool(name="attn", bufs=3) as apool,
                tc.tile_pool(name="hwork", bufs=4) as hwork,
                tc.tile_pool(name="ps_s", bufs=2, space="PSUM") as ps_s,
                tc.tile_pool(name="ps_o", bufs=2, space="PSUM") as ps_o,
            ):
                kT_sb = gath.tile([P, G6, GROUP, S], f32r)      # gathered k'^T
                vaug_sb = gath.tile([P, MCH, H, DH + 1], bf16)  # gathered v + ones
                for g in range(GROUP):
                    eng = nc.sync if g % 2 == 0 else nc.scalar
                    eng.dma_start(
                        out=kT_sb[:, :, g, :],
                        in_=cc_out_k[g:g + 1, :].rearrange(
                            "x (s p n) -> (x p) s n", p=P, s=G6
                        ),
                    )
                for g in range(GROUP):
                    eng = nc.sync if g % 2 == 0 else nc.scalar
                    eng.dma_start(
                        out=vaug_sb[:, 4 * g:4 * g + 4, :, :].rearrange(
                            "p c h d -> p c (h d)"
                        ),
                        in_=cc_out_v[g:g + 1, :].rearrange(
                            "x (c p f) -> (x p) c f", c=NCH, p=P
                        ),
                    )
                for h in range(H):
                    p0 = DH * (h % 2)
                    grp = h // 2
                    po = ps_o.tile([DH + 1, S], f32)
                    for mc0, w in ((0, 3), (3, 3), (6, 3), (9, 3), (12, 3), (15, 1)):
                        ps = ps_s.tile([P, 3, S], f32, tag="ps3")
                        for j in range(w):
                            mc = mc0 + j
                            nc.tensor.matmul(
                                ps[:, j],
                                kT_sb[p0:p0 + DH, grp, mc // 4,
                                      (mc % 4) * P:(mc % 4 + 1) * P],
                                qT_sb[p0:p0 + DH, grp, :],
                                start=True, stop=True,
                            )
                        at = apool.tile([P, 3, S], bf16, tag="at")
                        nc.scalar.activation(
                            out=at[:, :w], in_=ps[:, :w], func=ACTF.Exp, scale=SCALE
                        )
                        for j in range(w):
                            mc = mc0 + j
                            nc.tensor.matmul(
                                po, vaug_sb[:, mc, h, :], at[:, j],
                                start=(mc == 0), stop=(mc == MCH - 1),
                            )
                    rinv = hwork.tile([1, S], f32, tag="rinv")
                    nc.vector.reciprocal(out=rinv, in_=po[DH:DH + 1, :])
                    rbc = hwork.tile([DH, S], f32, tag="rbc")
                    nc.gpsimd.partition_broadcast(rbc, rinv)
                    nc.vector.tensor_tensor(
                        out=outT_sb[p0:p0 + DH, grp, :], in0=po[0:DH, :],
                        in1=rbc, op=ALU.mult,
                    )

        # ---- output proj, gate, fuse, final LN ----
        with (
            tc.tile_pool(name="zpool", bufs=1) as zpool,
            tc.tile_pool(name="fwork", bufs=2) as fwork,
            tc.tile_pool(name="stat2", bufs=6) as stat2,
            tc.tile_pool(name="ps_z", bufs=2, space="PSUM") as ps_z,
            tc.tile_pool(name="ps_t2", bufs=2, space="PSUM") as ps_t2,
        ):
            vis_sb = zpool.tile([P, NCH, D], f32)
            nc.gpsimd.dma_start(
                out=vis_sb, in_=vis_nat.rearrange("(c p) o -> p c o", p=P)
            )
            gwT_sb = zpool.tile([P, 2 * G6, D], f32r)
            nc.scalar.dma_start(
                out=gwT_sb, in_=gwT.rearrange("(s p) o -> p s o", p=P)
            )
            z_sb = zpool.tile([P, NCH, D], f32)
            zT_sb = zpool.tile([P, G6, S], f32r)
            gbc = zpool.tile([P, D], f32)
            bbc = zpool.tile([P, D], f32)

            # broadcast final-LN gain/bias across partitions via K=1 matmul
            for dst, src_row in ((gbc, lnfw_sb), (bbc, lnfb_sb)):
                pb = ps_z.tile([P, D], f32, tag="pz")
                for o0, o1 in HALves:
                    nc.tensor.matmul(
                        pb[:, o0:o1], ones_f32, src_row[:, o0:o1],
                        start=True, stop=True,
                    )
                nc.vector.tensor_copy(out=dst, in_=pb)

            def ln_stats2(y):
                st = stat2.tile([P, 2, 6], f32)
                for i in range(2):
                    nc.vector.bn_stats(out=st[:, i], in_=y[:, i * 384:(i + 1) * 384])
                mv = stat2.tile([P, 2], f32)
                nc.vector.bn_aggr(out=mv, in_=st)
                rstd = stat2.tile([P, 1], f32)
                nc.scalar.activation(
                    out=rstd, in_=mv[:, 1:2], func=ACTF.Sqrt, bias=eps_t, scale=1.0
                )
                nc.vector.reciprocal(out=rstd, in_=rstd)
                negmr = stat2.tile([P, 1], f32)
                nc.vector.tensor_scalar(
                    out=negmr, in0=mv[:, 0:1], scalar1=rstd, scalar2=-1.0,
                    op0=ALU.mult, op1=ALU.mult,
                )
                return negmr, rstd

            for c in range(NCH):
                pz = ps_z.tile([P, D], f32, tag="pz")
                for o0, o1 in HALves:
                    for s in range(G6):
                        nc.tensor.matmul(
                            pz[:, o0:o1],
                            outT_sb[:, s, c * P:(c + 1) * P],
                            woT_sb[:, s, o0:o1],
                            start=(s == 0), stop=False,
                        )
                    nc.tensor.matmul(
                        pz[:, o0:o1], ones_r, bo_sb[:, o0:o1],
                        start=False, stop=True,
                    )
                nc.scalar.copy(out=z_sb[:, c], in_=pz)
            for s in range(G6):
                pt = ps_t2.tile([P, NCH, P], f32)
                for c in range(NCH):
                    nc.tensor.transpose(
                        pt[:, c], z_sb[:, c, s * P:(s + 1) * P], ident_f32
                    )
                nc.scalar.copy(
                    out=zT_sb[:, s, :], in_=pt.rearrange("p c n -> p (c n)")
                )

            gsigs = []
            for c in range(NCH):
                pg = ps_z.tile([P, D], f32, tag="pz")
                for o0, o1 in HALves:
                    for s in range(G6):
                        nc.tensor.matmul(
                            pg[:, o0:o1],
                            xvT_sb[:, s, c * P:(c + 1) * P],
                            gwT_sb[:, s, o0:o1],
                            start=(s == 0), stop=False,
                        )
                    for s in range(G6):
                        nc.tensor.matmul(
                            pg[:, o0:o1],
                            zT_sb[:, s, c * P:(c + 1) * P],
                            gwT_sb[:, G6 + s, o0:o1],
                            start=False, stop=False,
                        )
                    nc.tensor.matmul(
                        pg[:, o0:o1], ones_r, gb_sb[:, o0:o1],
                        start=False, stop=True,
                    )
                gsig = zpool.tile([P, D], bf16, tag=f"gsig{c}")
                nc.scalar.activation(out=gsig, in_=pg, func=ACTF.Sigmoid)
                gsigs.append(gsig)

            for c in range(NCH):
                gsig = gsigs[c]
                dvz = fwork.tile([P, D], f32, tag="dvz")
                nc.gpsimd.tensor_tensor(
                    out=dvz, in0=vis_sb[:, c], in1=z_sb[:, c], op=ALU.subtract
                )
                fus = fwork.tile([P, D], f32, tag="fus")
                nc.vector.tensor_tensor(out=fus, in0=gsig, in1=dvz, op=ALU.mult)
                nc.vector.tensor_tensor(out=fus, in0=fus, in1=z_sb[:, c], op=ALU.add)
                negmr, rstd = ln_stats2(fus)
                tnorm = fwork.tile([P, D], f32, tag="tnorm")
                nc.scalar.activation(
                    out=tnorm, in_=fus, func=ACTF.Identity, bias=negmr, scale=rstd
                )
                nc.vector.tensor_tensor(out=tnorm, in0=tnorm, in1=gbc, op=ALU.mult)
                nc.vector.tensor_tensor(out=tnorm, in0=tnorm, in1=bbc, op=ALU.add)
                nc.sync.dma_start(
                    out=out_rows.rearrange("(c p) o -> p c o", p=P)[:, c], in_=tnorm
                )

    nc.compile()
    return nc


def _prepare_in_maps(inputs):
    f32 = np.float32
    vis = np.asarray(inputs["visible_features"], f32)
    inf = np.asarray(inputs["infrared_features"], f32)
    wq = np.asarray(inputs["wq"], f32)
    bq = np.asarray(inputs["bq"], f32)
    lnq_w = np.asarray(inputs["lnq_w"], f32)
    lnq_b = np.asarray(inputs["lnq_b"], f32)
    wk = np.asarray(inputs["wk"], f32)
    bk = np.asarray(inputs["bk"], f32)
    lnk_w = np.asarray(inputs["lnk_w"], f32)
    lnk_b = np.asarray(inputs["lnk_b"], f32)
    wv = np.asarray(inputs["wv"], f32)
    bv = np.asarray(inputs["bv"], f32)
    lnv_w = np.asarray(inputs["lnv_w"], f32)
    lnv_b = np.asarray(inputs["lnv_b"], f32)
    pos = np.asarray(inputs["pos_emb"], f32)[:N]
    wo = np.asarray(inputs["wo"], f32)
    bo = np.asarray(inputs["bo"], f32)
    gw = np.asarray(inputs["gate_w"], f32)
    gb_ = np.asarray(inputs["gate_b"], f32)
    ln_w = np.asarray(inputs["ln_w"], f32)
    ln_b = np.asarray(inputs["ln_b"], f32)

    wqkvT = np.ascontiguousarray(np.concatenate([wq.T, wk.T, wv.T], axis=1))
    bqkv = np.ascontiguousarray(np.concatenate([bq, bk, bv])[None])
    woT = np.ascontiguousarray((wo * lnv_w[None, :]).T)   # fold LN_v gain
    bo_a = np.ascontiguousarray((bo + wo @ lnv_b)[None])  # fold LN_v bias
    gwT = np.ascontiguousarray(gw.T)
    gbr = np.ascontiguousarray(gb_[None])
    lnq_g = np.ascontiguousarray(lnq_w.reshape(G6, P).T)
    lnq_b2 = np.ascontiguousarray(lnq_b.reshape(G6, P).T)
    lnk_g = np.ascontiguousarray(lnk_w.reshape(G6, P).T)
    lnf = np.stack([ln_w, ln_b])

    in_maps = []
    for c in range(CORES):
        b, r0 = c // GROUP, (c % GROUP) * S
        in_maps.append({
            "xqT": np.ascontiguousarray(inf[b, r0:r0 + S].T),
            "xvT": np.ascontiguousarray(vis[b, r0:r0 + S].T),
            "vis_nat": np.ascontiguousarray(vis[b, r0:r0 + S]),
            "posTb": np.ascontiguousarray(
                pos[r0:r0 + S].T / SCALE + lnk_b[:, None]
            ).astype(BF),
            "wqkvT": wqkvT,
            "bqkv": bqkv,
            "woT": woT,
            "bo_a": bo_a,
            "gwT": gwT,
            "gb": gbr,
            "lnq_g": lnq_g,
            "lnq_b": lnq_b2,
            "lnk_g": lnk_g,
            "lnf": lnf,
        })
    return in_maps


def kernel(trace=False, **inputs):
    from concourse.bass_utils import run_bass_kernel_spmd

    if "nc" not in _CACHE:
        _CACHE["nc"] = _build()
    nc = _CACHE["nc"]
    in_maps = _prepare_in_maps(inputs)
    res = run_bass_kernel_spmd(
        nc, in_maps, core_ids=list(range(CORES)), trace=trace
    )
    out = np.empty((B, N, D), np.float32)
    for c in range(CORES):
        b, r0 = c // GROUP, (c % GROUP) * S
        out[b, r0:r0 + S] = res.results[c]["out_rows"]
    _CACHE["last_result"] = res
    return out



# revision 30
# speedup vs baseline: 1.1664x; 1.1664x over previous
"""Trainium2 Bass kernel for nn_CustomCrossModalAttention (B=2, N=2048, D=768, H=12).

Sharding v2 (8 cores, head-parallel attention, minimal collective traffic):
  - core c owns batch b = c//4 and heads [3*(c%4), 3*(c%4)+3) => a 192-wide
    column slice of the projection output, for ALL 2048 rows of its batch.
  - Phase 1 (column-parallel): q,k,v projections computed directly in the
    TRANSPOSED layout yT[cols, rows] = W_slice @ x^T (no PE transposes for
    q/k). LayerNorm row stats (sum, sumsq over all 768 cols) are partial:
    each core has 192 cols, so partial sums are exchanged with ONE small
    AllGather ([6,2048] f32 per core -> 197KB) and summed locally.
    Row sums come FREE from an extra weight column (w_sum = sum_j w_j)
    appended to the 64-wide B-tile matmul (M=65). Sumsqs via ones-matmuls
    over squared tiles.
  - q,k normalized in transposed layout (mu/rstd broadcast along partitions
    via K=1 ones-matmuls); k is folded with pos/scale + lnk_b (host-side).
    v is PE-transposed to natural [keys, cols] layout with the normalize
    fused into the PSUM->SBUF move (per-partition scalars).
  - Attention: 3 heads x 2048 q rows x 2048 keys per core, all local.
    Softmax denominator via a ones column appended to v (M=65).
  - Output projection is computed as per-core PARTIALS z_c = outT_c^T @
    woT[slice] for all 2048 rows, exchanged with ONE ReduceScatter(add)
    (bf16, out 512x768 = 786KB) that also returns to row-sharding.
  - Epilogue (row-parallel, 512 own rows): gate, fuse, final LayerNorm --
    same structure as the row-sharded kernel. gate's vis-half matmuls and
    LN broadcasts run during the ReduceScatter window.

Collectives total: ~20us (stats AG) + ~35us (z RS) vs 267us of k/v
AllGathers in the row-sharded version.
"""

import numpy as np
import ml_dtypes

B, N, D = 2, 2048, 768
H, DH = 12, 64
P = 128
CORES, GROUP = 8, 4
HPC = 3                 # heads per core
HP = HPC * DH           # 192 cols per core
S = 512                 # own rows per core (epilogue)
NCH = N // P            # 16 row/key chunks
QCH = N // S            # 4 q blocks of 512
G6 = D // P             # 6
SCALE = DH ** -0.5
EPS = 1e-5
RD = 1.0 / D

BF = ml_dtypes.bfloat16

_CACHE = {}


def _build():
    from contextlib import ExitStack

    import concourse.bacc as bacc
    import concourse.mybir as mybir
    import concourse.tile as tile
    from concourse.masks import make_identity

    f32 = mybir.dt.float32
    f32r = mybir.dt.float32r
    bf16 = mybir.dt.bfloat16
    ALU = mybir.AluOpType
    ACTF = mybir.ActivationFunctionType

    nc = bacc.Bacc("TRN2", target_bir_lowering=False, num_devices=CORES)

    def din(name, shape, dt=bf16):
        return nc.dram_tensor(name, shape, dt, kind="ExternalInput")

    # ---- inputs ----
    xqT = din("xqT", [D, N], f32r)        # infrared^T, full batch rows
    xvT = din("xvT", [D, N], f32r)        # visible^T, full batch rows
    xvTown = din("xvTown", [D, S], f32r)  # visible^T, own 512 rows (gate)
    vis_nat = din("vis_nat", [S, D], f32)
    wA = din("wA", [D, 3 * P], f32r)      # per-tensor A cols (128) of W^T slice
    wB = din("wB", [D, 3 * 65], f32r)     # per-tensor B cols (64) + w_sum col
    biasA = din("biasA", [P, 3], f32)
    biasB = din("biasB", [65, 3], f32)
    posTbbA = din("posTbbA", [P, N])      # (pos/scale + lnk_b)^T slice, A part
    posTbbB = din("posTbbB", [DH, N])     # B part (head 2)
    lnqA = din("lnqA", [P, 2], f32)       # lnq gain, bias (A cols)
    lnqB = din("lnqB", [DH, 2], f32)
    lnkA = din("lnkA", [P, 1], f32)       # lnk gain (bias folded into posTbb)
    lnkB = din("lnkB", [DH, 1], f32)
    woTsl = din("woTsl", [HP, D])         # (wo*lnv_w)^T rows for my 192 dims, bf16
    bo_a = din("bo_a", [1, D], f32r)      # bo + wo @ lnv_b
    gwvT = din("gwvT", [D, D], f32r)      # gate weight, vis half, transposed
    gwzT = din("gwzT", [D, D])            # gate weight, z half, transposed (bf16)
    gb = din("gb", [1, D], f32r)
    lnf = din("lnf", [2, D], f32)
    sumb = din("sumb", [12, 1], f32)      # sum(b_slice)/D per (t, chunk) row
    out_rows = nc.dram_tensor("out_rows", [S, D], f32, kind="ExternalOutput")
    dbg = nc.dram_tensor("dbg", [4, 12 * S], f32, kind="ExternalOutput")

    # ---- collective buffers ----
    cc_in_s = nc.dram_tensor("cc_in_s", [6 * N], f32)
    srow_dram = nc.dram_tensor("srow_dram", [2, 12 * S], f32)
    cc_out_s = nc.dram_tensor("cc_out_s", [GROUP, 6 * N], f32)
    cc_in_z = nc.dram_tensor("cc_in_z", [N * D], bf16)
    cc_out_z = nc.dram_tensor("cc_out_z", [S * D], bf16)
    groups = [[0, 1, 2, 3], [4, 5, 6, 7]]

    HALves = [(0, 512), (512, D)]

    with tile.TileContext(nc) as tc, ExitStack() as ctx:
        const = ctx.enter_context(tc.tile_pool(name="const", bufs=1))
        mid_cm = tc.tile_pool(name="mid", bufs=1)
        persist = mid_cm.__enter__()

        ident = const.tile([P, P], f32)
        make_identity(nc, ident)
        ones_c_f = const.tile([P, 1], f32)
        nc.vector.memset(ones_c_f, 1.0)
        ones_c = ones_c_f.bitcast(f32r)
        ones_r_f = const.tile([1, P], f32)
        nc.vector.memset(ones_r_f, 1.0)
        ones_r = ones_r_f.bitcast(f32r)
        ones_bf = const.tile([P, 1], bf16)
        nc.vector.memset(ones_bf, 1.0)
        eps12 = const.tile([12, 1], f32)
        nc.vector.memset(eps12, EPS * RD * RD * 0.0 + EPS)
        eps_t = const.tile([P, 1], f32)
        nc.vector.memset(eps_t, EPS)

        # persistent across phases
        qTA = persist.tile([P, N], f32r)       # heads 0,1 dims on partitions
        qTB = persist.tile([DH, N], f32r)      # head 2
        kTA = persist.tile([P, N], f32r)
        kTB = persist.tile([DH, N], f32r)
        v_aug = persist.tile([P, NCH, HPC, DH + 1], bf16)
        nc.vector.memset(v_aug[:, :, :, DH:DH + 1], 1.0)
        st = persist.tile([12, 2, GROUP, S], f32)  # [(t,c4) | kind, g, 512]
        vstd = persist.tile([P, 2, NCH], f32)     # v rstd / negmurstd per key row

        # ---------------- phase 1: projections + stats ----------------
        with tc.tile_pool(name="ph1", bufs=1) as ph1:
            psproj_cm = ExitStack()
            psA_p = psproj_cm.enter_context(
                tc.tile_pool(name="psA", bufs=2, space="PSUM"))
            psB_p = psproj_cm.enter_context(
                tc.tile_pool(name="psB", bufs=2, space="PSUM"))
            psS_p = psproj_cm.enter_context(
                tc.tile_pool(name="psS", bufs=2, space="PSUM"))
            yA = [ph1.tile([P, N], f32, name=f"yA{t}") for t in range(3)]
            yBp = ph1.tile([P, N], f32)   # packed: q at parts 0:64, k at 64:128
            yB2 = ph1.tile([DH, N], f32)  # v
            yB = [(yBp, 0), (yBp, DH), (yB2, 0)]
            posTbbA_sb = ph1.tile([P, N], bf16)
            nc.scalar.dma_start(out=posTbbA_sb, in_=posTbbA.ap())
            posTbbB_sb = ph1.tile([DH, N], bf16)
            nc.scalar.dma_start(out=posTbbB_sb, in_=posTbbB.ap())
            lnqA_sb = ph1.tile([P, 2], f32)
            nc.sync.dma_start(out=lnqA_sb, in_=lnqA.ap())
            lnqB_sb = ph1.tile([DH, 2], f32)
            nc.sync.dma_start(out=lnqB_sb, in_=lnqB.ap())
            lnkA_sb = ph1.tile([P, 1], f32)
            nc.sync.dma_start(out=lnkA_sb, in_=lnkA.ap())
            lnkB_sb = ph1.tile([DH, 1], f32)
            nc.sync.dma_start(out=lnkB_sb, in_=lnkB.ap())

            def project(t, rhs, sq_pool, srow_pool):
                yBt, b0 = yB[t]
                for c in range(QCH):
                    n0 = c * S
                    pa = psA_p.tile([P, S], f32, tag="pa")
                    for s in range(G6):
                        nc.tensor.matmul(
                            pa, wA_sb[:, s, t * P:(t + 1) * P],
                            rhs[:, s, n0:n0 + S],
                            start=(s == 0), stop=(s == G6 - 1),
                        )
                    pb = psB_p.tile([65, S], f32, tag="pb")
                    for s in range(G6):
                        nc.tensor.matmul(
                            pb, wB_sb[:, s, t * 65:(t + 1) * 65],
                            rhs[:, s, n0:n0 + S],
                            start=(s == 0), stop=(s == G6 - 1),
                        )
                    nc.vector.tensor_scalar(
                        out=yA[t][:, n0:n0 + S], in0=pa,
                        scalar1=biasA_sb[:, t:t + 1], scalar2=None, op0=ALU.add,
                    )
                    nc.scalar.activation(
                        out=yBt[b0:b0 + DH, n0:n0 + S], in_=pb[0:DH],
                        func=ACTF.Identity, bias=biasB_sb[0:DH, t:t + 1],
                        scale=1.0,
                    )
                    srow = srow_pool.tile([1, S], f32, tag="srow")
                    nc.scalar.copy(out=srow, in_=pb[DH:DH + 1])
                    nc.sync.dma_start(
                        out=cc_in_s.ap().rearrange("(r n) -> r n", r=6)[
                            t:t + 1, n0:n0 + S],
                        in_=srow,
                    )
                ysqA = sq_pool.tile([P, N], bf16, tag="ysqA")
                ysqB = sq_pool.tile([DH, N], bf16, tag="ysqB")
                nc.vector.tensor_tensor(out=ysqA, in0=yA[t], in1=yA[t], op=ALU.mult)
                nc.gpsimd.tensor_tensor(out=ysqB, in0=yBt[b0:b0 + DH],
                                        in1=yBt[b0:b0 + DH], op=ALU.mult)
                for c in range(QCH):
                    n0 = c * S
                    pq = psS_p.tile([1, S], f32, tag="pq")
                    nc.tensor.matmul(pq, ones_bf, ysqA[:, n0:n0 + S],
                                     start=True, stop=False)
                    nc.tensor.matmul(pq, ones_bf[0:DH], ysqB[:, n0:n0 + S],
                                     start=False, stop=True)
                    qrow = srow_pool.tile([1, S], f32, tag="qrow")
                    nc.scalar.copy(out=qrow, in_=pq)
                    nc.gpsimd.dma_start(
                        out=cc_in_s.ap().rearrange("(r n) -> r n", r=6)[
                            3 + t:4 + t, n0:n0 + S],
                        in_=qrow,
                    )

            with (
                tc.tile_pool(name="ph1w", bufs=1) as ph1w,
                tc.tile_pool(name="ph1sq", bufs=2) as ph1sq,
                tc.tile_pool(name="strow", bufs=4) as strow,
            ):
                wA_sb = ph1w.tile([P, G6, 3 * P], f32r)
                nc.sync.dma_start(out=wA_sb,
                                  in_=wA.rearrange("(s p) o -> p s o", p=P))
                wB_sb = ph1w.tile([P, G6, 3 * 65], f32r)
                nc.scalar.dma_start(out=wB_sb,
                                    in_=wB.rearrange("(s p) o -> p s o", p=P))
                biasA_sb = ph1w.tile([P, 3], f32)
                nc.sync.dma_start(out=biasA_sb, in_=biasA.ap())
                biasB_sb = ph1w.tile([65, 3], f32)
                nc.sync.dma_start(out=biasB_sb, in_=biasB.ap())

                with tc.tile_pool(name="ph1v", bufs=1) as ph1v:
                    xvT_sb = ph1v.tile([P, G6, N], f32r)
                    nc.gpsimd.dma_start(
                        out=xvT_sb[:, 0:3, :],
                        in_=xvT.rearrange("(s p) n -> p s n", p=P)[:, 0:3, :])
                    nc.sync.dma_start(
                        out=xvT_sb[:, 3:6, :],
                        in_=xvT.rearrange("(s p) n -> p s n", p=P)[:, 3:6, :])
                    project(1, xvT_sb, ph1sq, strow)
                    project(2, xvT_sb, ph1sq, strow)
                with tc.tile_pool(name="ph1q", bufs=1) as ph1q:
                    xqT_sb = ph1q.tile([P, G6, N], f32r)
                    nc.gpsimd.dma_start(
                        out=xqT_sb[:, 0:3, :],
                        in_=xqT.rearrange("(s p) n -> p s n", p=P)[:, 0:3, :])
                    nc.sync.dma_start(
                        out=xqT_sb[:, 3:6, :],
                        in_=xqT.rearrange("(s p) n -> p s n", p=P)[:, 3:6, :])
                    project(0, xqT_sb, ph1sq, strow)

            # kick stats AllGather
            nc.gpsimd.collective_compute(
                "AllGather", ALU.bypass, replica_groups=groups,
                ins=[cc_in_s.ap().opt()], outs=[cc_out_s.ap().opt()],
            )
            psproj_cm.close()

            # gathered stats -> combined mu/rstd
            ph1n_cm = tc.tile_pool(name="ph1n", bufs=1)
            ph1n = ph1n_cm.__enter__()
            sumb_sb = ph1.tile([12, 1], f32)
            nc.sync.dma_start(out=sumb_sb, in_=sumb.ap())
            for k in range(2):
                nc.sync.dma_start(
                    out=st[:, k],
                    in_=cc_out_s.ap()[:, k * 6 * N // 2:(k + 1) * 6 * N // 2]
                    .rearrange("g (t c i) -> (t c) g i", t=3, c=QCH),
                )
            # sum over the 4 group shards (free-dim slices, all base 0)
            nc.vector.tensor_tensor(out=st[:, :, 0, :], in0=st[:, :, 0, :],
                                    in1=st[:, :, 1, :], op=ALU.add)
            nc.vector.tensor_tensor(out=st[:, :, 2, :], in0=st[:, :, 2, :],
                                    in1=st[:, :, 3, :], op=ALU.add)
            nc.vector.tensor_tensor(out=st[:, :, 0, :], in0=st[:, :, 0, :],
                                    in1=st[:, :, 2, :], op=ALU.add)
            mu = ph1n.tile([12, S], f32)      # rows (t,c4): mu
            rstd = ph1n.tile([12, S], f32r)
            nmr = ph1n.tile([12, S], f32r)    # -mu*rstd
            nc.vector.tensor_scalar(out=mu, in0=st[:, 0, 0, :], scalar1=RD,
                                    scalar2=sumb_sb, op0=ALU.mult, op1=ALU.add)
            with nc.allow_low_precision(reason="f32r rstd/nmr for bcast matmuls"):
                nc.vector.tensor_scalar(out=rstd, in0=st[:, 1, 0, :], scalar1=RD,
                                        scalar2=None, op0=ALU.mult)  # E[y^2]
                nc.vector.tensor_tensor(out=nmr, in0=mu, in1=mu, op=ALU.mult)
                nc.vector.tensor_tensor(out=rstd, in0=rstd, in1=nmr,
                                        op=ALU.subtract)
                nc.scalar.activation(out=rstd, in_=rstd, func=ACTF.Sqrt,
                                     bias=eps12, scale=1.0)
                nc.vector.reciprocal(out=rstd, in_=rstd)
                nc.vector.tensor_tensor(out=nmr, in0=mu, in1=rstd, op=ALU.mult)
                nc.vector.tensor_scalar(out=nmr, in0=nmr, scalar1=-1.0,
                                        scalar2=None, op0=ALU.mult)
            nc.scalar.dma_start(out=dbg.ap()[0:1].rearrange("x (r n) -> (x r) n", r=12),
                                in_=st[:, 0, 0, :])
            nc.scalar.dma_start(out=dbg.ap()[1:2].rearrange("x (r n) -> (x r) n", r=12),
                                in_=st[:, 1, 0, :])
            # consolidate to single-partition rows (matmul rhs needs base
            # partition 0; free-dim slices are always legal)
            rstd_row = ph1n.tile([1, 12 * S], f32r)
            nmr_row = ph1n.tile([1, 12 * S], f32r)
            nc.sync.dma_start(
                out=srow_dram.ap()[0:1].rearrange("x (r n) -> (x r) n", r=12),
                in_=rstd.bitcast(f32))
            nc.sync.dma_start(
                out=srow_dram.ap()[1:2].rearrange("x (r n) -> (x r) n", r=12),
                in_=nmr.bitcast(f32))
            nc.sync.dma_start(out=rstd_row.bitcast(f32), in_=srow_dram.ap()[0:1])
            nc.sync.dma_start(out=nmr_row.bitcast(f32), in_=srow_dram.ap()[1:2])
            nc.scalar.dma_start(out=dbg.ap()[2:3], in_=rstd_row.bitcast(f32))
            nc.scalar.dma_start(out=dbg.ap()[3:4], in_=nmr_row.bitcast(f32))

            # v per-key-row scalars: DMA-transpose rstd/nmr rows (t=2) ->
            # [128, 16] each
            for r in range(2):
                nc.sync.dma_start(
                    out=vstd[:, r, :],
                    in_=srow_dram.ap()[r:r + 1, 8 * S:12 * S].rearrange(
                        "x (c j p) -> (x p) (c j)", p=P, c=QCH),
                )

            # normalize q,k in transposed layout
            with tc.tile_pool(name="psBC", bufs=2, space="PSUM") as psBC:
                for t in range(2):
                    gA = lnqA_sb[:, 0:1] if t == 0 else lnkA_sb
                    gB = lnqB_sb[:, 0:1] if t == 0 else lnkB_sb
                    oA = qTA if t == 0 else kTA
                    oB = qTB if t == 0 else kTB
                    for c in range(QCH):
                        n0 = c * S
                        pbc = psBC.tile([P, 2, S], f32, tag="pbc")
                        i0 = (4 * t + c) * S
                        nc.tensor.matmul(pbc[:, 0], ones_r,
                                         rstd_row[:, i0:i0 + S],
                                         start=True, stop=True)
                        nc.tensor.matmul(pbc[:, 1], ones_r,
                                         nmr_row[:, i0:i0 + S],
                                         start=True, stop=True)
                        yBt, b0 = yB[t]
                        tA = ph1n.tile([P, S], f32, tag="tA")
                        tB = ph1n.tile([DH, S], f32, tag="tB")
                        nc.vector.tensor_tensor(out=tA, in0=yA[t][:, n0:n0 + S],
                                                in1=pbc[:, 0], op=ALU.mult)
                        nc.vector.tensor_tensor(out=tB,
                                                in0=yBt[b0:b0 + DH, n0:n0 + S],
                                                in1=pbc[0:DH, 0], op=ALU.mult)
                        nc.vector.tensor_tensor(out=tA, in0=tA, in1=pbc[:, 1],
                                                op=ALU.add)
                        nc.vector.tensor_tensor(out=tB, in0=tB, in1=pbc[0:DH, 1],
                                                op=ALU.add)
                        if t == 0:
                            nc.scalar.activation(
                                out=oA[:, n0:n0 + S], in_=tA, func=ACTF.Identity,
                                scale=gA, bias=lnqA_sb[:, 1:2],
                            )
                            nc.scalar.activation(
                                out=oB[:, n0:n0 + S], in_=tB, func=ACTF.Identity,
                                scale=gB, bias=lnqB_sb[:, 1:2],
                            )
                        else:
                            nc.vector.scalar_tensor_tensor(
                                out=oA[:, n0:n0 + S], in0=tA,
                                scalar=gA, in1=posTbbA_sb[:, n0:n0 + S],
                                op0=ALU.mult, op1=ALU.add,
                            )
                            nc.vector.scalar_tensor_tensor(
                                out=oB[:, n0:n0 + S], in0=tB,
                                scalar=gB, in1=posTbbB_sb[:, n0:n0 + S],
                                op0=ALU.mult, op1=ALU.add,
                            )

                # v: transpose + normalize fused into the copy-out
                for kc in range(NCH):
                    ptv = psBC.tile([P, P + DH], f32, tag="vt")
                    nc.tensor.transpose(
                        ptv[:, 0:P], yA[2][:, kc * P:(kc + 1) * P], ident,
                    )
                    nc.tensor.transpose(
                        ptv[:, P:P + DH], yB2[:, kc * P:(kc + 1) * P],
                        ident[0:DH, 0:DH],
                    )
                    nc.scalar.activation(
                        out=v_aug[:, kc, :, 0:DH],
                        in_=ptv, func=ACTF.Identity,
                        bias=vstd[:, 1, kc:kc + 1], scale=vstd[:, 0, kc:kc + 1],
                    )
            ph1n_cm.__exit__(None, None, None)

        # ---------------- attention + z partials ----------------
        with (
            tc.tile_pool(name="attn", bufs=1) as ap_,
            tc.tile_pool(name="atp", bufs=2) as atp,
            tc.tile_pool(name="hw", bufs=4) as hw,
            tc.tile_pool(name="ps_s", bufs=2, space="PSUM") as ps_s,
            tc.tile_pool(name="ps_o", bufs=1, space="PSUM") as ps_o,
            tc.tile_pool(name="ps_z", bufs=1, space="PSUM") as ps_z,
        ):
            woT_sb = ap_.tile([P, 2, D], bf16)   # [A(128) | B pad] rows of woT slice
            nc.sync.dma_start(out=woT_sb[:, 0, :], in_=woTsl.ap()[0:P])
            nc.sync.dma_start(out=woT_sb[0:DH, 1, :], in_=woTsl.ap()[P:HP])
            outTA = ap_.tile([P, N], bf16)
            outTB = ap_.tile([DH, N], bf16)
            zparts = ap_.tile([P, NCH, D], bf16)

            KCG = [(0, 3), (3, 3), (6, 3), (9, 3), (12, 3), (15, 1)]
            for qc in range(QCH):
                n0 = qc * S
                for h in range(HPC):
                    kT = [kTA[0:DH], kTA[DH:P], kTB][h]
                    qT = [qTA[0:DH], qTA[DH:P], qTB][h]
                    oT = [outTA[0:DH], outTA[DH:P], outTB][h]
                    po = ps_o.tile([DH + 1, S], f32, tag="po")
                    for mc0, w in KCG:
                        ps = ps_s.tile([P, 3, S], f32, tag="ps3")
                        for j in range(w):
                            mc = mc0 + j
                            nc.tensor.matmul(
                                ps[:, j], kT[:, mc * P:(mc + 1) * P],
                                qT[:, n0:n0 + S], start=True, stop=True,
                            )
                        at = atp.tile([P, 3, S], bf16, tag="at")
                        nc.scalar.activation(out=at[:, :w], in_=ps[:, :w],
                                             func=ACTF.Exp, scale=SCALE)
                        for j in range(w):
                            mc = mc0 + j
                            nc.tensor.matmul(
                                po, v_aug[:, mc, h, :], at[:, j],
                                start=(mc == 0), stop=(mc == NCH - 1),
                            )
                    rinv = hw.tile([1, S], f32, tag="rinv")
                    nc.vector.reciprocal(out=rinv, in_=po[DH:DH + 1, :])
                    rbc = hw.tile([DH, S], f32, tag="rbc")
                    nc.gpsimd.partition_broadcast(rbc, rinv)
                    nc.vector.tensor_tensor(out=oT[:, n0:n0 + S], in0=po[0:DH, :],
                                            in1=rbc, op=ALU.mult)
                # z partials for this q block (rows n0..n0+512)
                for rc in range(4):
                    r0 = n0 + rc * P
                    ch = qc * 4 + rc
                    for o0, o1 in HALves:
                        pz = ps_z.tile([P, S], f32, tag="pz")
                        nc.tensor.matmul(pz[:, 0:o1 - o0], outTA[:, r0:r0 + P],
                                         woT_sb[:, 0, o0:o1], start=True, stop=False)
                        nc.tensor.matmul(pz[:, 0:o1 - o0], outTB[:, r0:r0 + P],
                                         woT_sb[0:DH, 1, o0:o1], start=False,
                                         stop=True)
                        if (ch + (o0 > 0)) % 2 == 0:
                            nc.vector.tensor_copy(out=zparts[:, ch, o0:o1],
                                                  in_=pz[:, 0:o1 - o0])
                        else:
                            nc.scalar.copy(out=zparts[:, ch, o0:o1],
                                           in_=pz[:, 0:o1 - o0])
            nc.sync.dma_start(
                out=cc_in_z.ap().rearrange("(c p o) -> p c o", p=P, c=NCH, o=D),
                in_=zparts,
            )
            nc.gpsimd.collective_compute(
                "ReduceScatter", ALU.add, replica_groups=groups,
                ins=[cc_in_z.ap().opt()], outs=[cc_out_z.ap().opt()],
            )

        mid_cm.__exit__(None, None, None)

        # ---------------- epilogue: gate, fuse, final LN ----------------
        with (
            tc.tile_pool(name="epi", bufs=1) as epi,
            tc.tile_pool(name="fwork", bufs=2) as fwork,
            tc.tile_pool(name="stat2", bufs=6) as stat2,
            tc.tile_pool(name="ps_g", bufs=2, space="PSUM") as ps_g,
            tc.tile_pool(name="ps_bc", bufs=1, space="PSUM") as ps_bc,
            tc.tile_pool(name="ps_t2", bufs=2, space="PSUM") as ps_t2,
        ):
            # loads + vis-half gate matmuls + LN broadcasts run during the RS
            xvo_sb = epi.tile([P, G6, S], f32r)
            nc.gpsimd.dma_start(out=xvo_sb,
                                in_=xvTown.rearrange("(s p) n -> p s n", p=P))
            gwv_sb = epi.tile([P, G6, D], f32r)
            nc.scalar.dma_start(out=gwv_sb,
                                in_=gwvT.rearrange("(s p) o -> p s o", p=P))
            gwz_sb = epi.tile([P, G6, D], bf16)
            nc.scalar.dma_start(out=gwz_sb,
                                in_=gwzT.rearrange("(s p) o -> p s o", p=P))
            vis_sb = epi.tile([P, QCH, D], f32)
            nc.scalar.dma_start(out=vis_sb,
                                in_=vis_nat.rearrange("(c p) o -> p c o", p=P))
            bo_sb = epi.tile([1, D], f32r)
            nc.sync.dma_start(out=bo_sb, in_=bo_a.ap())
            gb_sb = epi.tile([1, D], f32r)
            nc.sync.dma_start(out=gb_sb, in_=gb.ap())
            lnfw_sb = epi.tile([1, D], f32)
            nc.sync.dma_start(out=lnfw_sb, in_=lnf.ap()[0:1, :])
            lnfb_sb = epi.tile([1, D], f32)
            nc.sync.dma_start(out=lnfb_sb, in_=lnf.ap()[1:2, :])

            gvis = epi.tile([P, QCH, D], f32)
            for c in range(QCH):
                pg = ps_g.tile([P, D], f32, tag="pg")
                for o0, o1 in HALves:
                    for s in range(G6):
                        nc.tensor.matmul(
                            pg[:, o0:o1], xvo_sb[:, s, c * P:(c + 1) * P],
                            gwv_sb[:, s, o0:o1], start=(s == 0), stop=False,
                        )
                    nc.tensor.matmul(pg[:, o0:o1], ones_r, gb_sb[:, o0:o1],
                                     start=False, stop=True)
                if c % 2 == 0:
                    nc.vector.tensor_copy(out=gvis[:, c], in_=pg)
                else:
                    nc.scalar.copy(out=gvis[:, c], in_=pg)

            # broadcasts: bo_a, lnf gain/bias
            gbc = epi.tile([P, D], f32)
            bbc = epi.tile([P, D], f32)
            bobc = epi.tile([P, D], f32)
            for dst, srow, one in ((gbc, lnfw_sb, ones_r_f),
                                   (bbc, lnfb_sb, ones_r_f),
                                   (bobc, bo_sb, ones_r)):
                pb = ps_bc.tile([P, D], f32, tag="pbb")
                for o0, o1 in HALves:
                    nc.tensor.matmul(pb[:, o0:o1], one, srow[:, o0:o1],
                                     start=True, stop=True)
                nc.scalar.copy(out=dst, in_=pb)

            # z from ReduceScatter (+ bias), transpose for the gate matmul
            z_sb = epi.tile([P, QCH, D], f32)
            zraw = epi.tile([P, QCH, D], bf16)
            nc.sync.dma_start(
                out=zraw, in_=cc_out_z.ap().rearrange("(c p o) -> p c o", p=P, c=QCH),
            )
            for c in range(QCH):
                e = nc.vector if c % 2 == 0 else nc.gpsimd
                e.tensor_tensor(out=z_sb[:, c], in0=zraw[:, c], in1=bobc,
                                op=ALU.add)
            zT_sb = epi.tile([P, G6, S], bf16)
            for s in range(G6):
                pt = ps_t2.tile([P, QCH, P], f32, tag="pt2")
                for c in range(QCH):
                    nc.tensor.transpose(
                        pt[:, c], z_sb[:, c, s * P:(s + 1) * P], ident,
                    )
                nc.scalar.copy(out=zT_sb[:, s, :],
                               in_=pt.rearrange("p c n -> p (c n)"))

            def ln_stats2(y):
                stt = stat2.tile([P, 2, 6], f32)
                for i in range(2):
                    nc.vector.bn_stats(out=stt[:, i], in_=y[:, i * 384:(i + 1) * 384])
                mv = stat2.tile([P, 2], f32)
                nc.vector.bn_aggr(out=mv, in_=stt)
                rstd2 = stat2.tile([P, 1], f32)
                nc.scalar.activation(out=rstd2, in_=mv[:, 1:2], func=ACTF.Sqrt,
                                     bias=eps_t, scale=1.0)
                nc.vector.reciprocal(out=rstd2, in_=rstd2)
                negmr = stat2.tile([P, 1], f32)
                nc.vector.tensor_scalar(out=negmr, in0=mv[:, 0:1], scalar1=rstd2,
                                        scalar2=-1.0, op0=ALU.mult, op1=ALU.mult)
                return negmr, rstd2

            gsigs = []
            for c in range(QCH):
                pg = ps_g.tile([P, D], f32, tag="pg")
                for o0, o1 in HALves:
                    for s in range(G6):
                        nc.tensor.matmul(
                            pg[:, o0:o1], zT_sb[:, s, c * P:(c + 1) * P],
                            gwz_sb[:, s, o0:o1], start=(s == 0), stop=(s == G6 - 1),
                        )
                gin = fwork.tile([P, D], f32, tag="gin")
                nc.vector.tensor_tensor(out=gin, in0=pg, in1=gvis[:, c], op=ALU.add)
                gsig = epi.tile([P, D], bf16, tag=f"gs{c}")
                nc.scalar.activation(out=gsig, in_=gin, func=ACTF.Sigmoid)
                gsigs.append(gsig)

            for c in range(QCH):
                dvz = fwork.tile([P, D], f32, tag="dvz")
                nc.gpsimd.tensor_tensor(out=dvz, in0=vis_sb[:, c], in1=z_sb[:, c],
                                        op=ALU.subtract)
                fus = fwork.tile([P, D], f32, tag="fus")
                nc.vector.tensor_tensor(out=fus, in0=gsigs[c], in1=dvz, op=ALU.mult)
                nc.vector.tensor_tensor(out=fus, in0=fus, in1=z_sb[:, c], op=ALU.add)
                negmr, rstd2 = ln_stats2(fus)
                tnorm = fwork.tile([P, D], f32, tag="tnorm")
                nc.scalar.activation(out=tnorm, in_=fus, func=ACTF.Identity,
                                     bias=negmr, scale=rstd2)
                nc.vector.tensor_tensor(out=tnorm, in0=tnorm, in1=gbc, op=ALU.mult)
                nc.vector.tensor_tensor(out=tnorm, in0=tnorm, in1=bbc, op=ALU.add)
                nc.sync.dma_start(
                    out=out_rows.rearrange("(c p) o -> p c o", p=P)[:, c],
                    in_=tnorm,
                )

    nc.compile()
    return nc


def _prepare_in_maps(inputs):
    f32 = np.float32
    vis = np.asarray(inputs["visible_features"], f32)
    inf = np.asarray(inputs["infrared_features"], f32)
    wq = np.asarray(inputs["wq"], f32)
    bq = np.asarray(inputs["bq"], f32)
    lnq_w = np.asarray(inputs["lnq_w"], f32)
    lnq_b = np.asarray(inputs["lnq_b"], f32)
    wk = np.asarray(inputs["wk"], f32)
    bk = np.asarray(inputs["bk"], f32)
    lnk_w = np.asarray(inputs["lnk_w"], f32)
    lnk_b = np.asarray(inputs["lnk_b"], f32)
    wv = np.asarray(inputs["wv"], f32)
    bv = np.asarray(inputs["bv"], f32)
    lnv_w = np.asarray(inputs["lnv_w"], f32)
    lnv_b = np.asarray(inputs["lnv_b"], f32)
    pos = np.asarray(inputs["pos_emb"], f32)[:N]
    wo = np.asarray(inputs["wo"], f32)
    bo = np.asarray(inputs["bo"], f32)
    gw = np.asarray(inputs["gate_w"], f32)
    gb_ = np.asarray(inputs["gate_b"], f32)
    ln_w = np.asarray(inputs["ln_w"], f32)
    ln_b = np.asarray(inputs["ln_b"], f32)

    woT = (wo * lnv_w[None, :]).T                       # [768(in=attn dims), 768]
    bo_a = np.ascontiguousarray((bo + wo @ lnv_b)[None])
    gwvT = np.ascontiguousarray(gw[:, 0:D].T)
    gwzT = np.ascontiguousarray(gw[:, D:2 * D].T).astype(BF)
    gbr = np.ascontiguousarray(gb_[None])
    lnf = np.stack([ln_w, ln_b])

    Ws = [wq, wk, wv]
    bs = [bq, bk, bv]

    in_maps = []
    for c in range(CORES):
        b, g = c // GROUP, c % GROUP
        hs = slice(g * HP, (g + 1) * HP)
        r0 = g * S
        wA = np.empty((D, 3 * P), f32)
        wB = np.empty((D, 3 * 65), f32)
        biasA = np.empty((P, 3), f32)
        biasB = np.empty((65, 3), f32)
        for t in range(3):
            WT = Ws[t].T[:, hs]          # [768, 192]
            wA[:, t * P:(t + 1) * P] = WT[:, 0:P]
            wB[:, t * 65:t * 65 + DH] = WT[:, P:HP]
            wB[:, t * 65 + DH] = WT.sum(axis=1)   # partial w_sum (own 192 cols)
            biasA[:, t] = bs[t][hs][0:P]
            biasB[0:DH, t] = bs[t][hs][P:HP]
            biasB[DH, t] = bs[t][hs].sum()
        posT = pos.T[hs] / SCALE + lnk_b[hs][:, None]  # [192, 2048]
        in_maps.append({
            "xqT": np.ascontiguousarray(inf[b].T),
            "xvT": np.ascontiguousarray(vis[b].T),
            "xvTown": np.ascontiguousarray(vis[b, r0:r0 + S].T),
            "vis_nat": np.ascontiguousarray(vis[b, r0:r0 + S]),
            "wA": wA, "wB": wB, "biasA": biasA, "biasB": biasB,
            "posTbbA": np.ascontiguousarray(posT[0:P]).astype(BF),
            "posTbbB": np.ascontiguousarray(posT[P:HP]).astype(BF),
            "lnqA": np.ascontiguousarray(
                np.stack([lnq_w[hs][0:P], lnq_b[hs][0:P]], 1)),
            "lnqB": np.ascontiguousarray(
                np.stack([lnq_w[hs][P:HP], lnq_b[hs][P:HP]], 1)),
            "lnkA": np.ascontiguousarray(lnk_w[hs][0:P, None]),
            "lnkB": np.ascontiguousarray(lnk_w[hs][P:HP, None]),
            "woTsl": np.ascontiguousarray(woT[hs]).astype(BF),
            "bo_a": bo_a,
            "gwvT": gwvT, "gwzT": gwzT, "gb": gbr,
            "lnf": lnf,
            "sumb": np.ascontiguousarray(
                np.repeat([bs[t].sum() / D for t in range(3)], 4)
            ).astype(f32).reshape(12, 1),
        })
    return in_maps


def kernel(trace=False, **inputs):
    from concourse.bass_utils import run_bass_kernel_spmd

    if "nc" not in _CACHE:
        _CACHE["nc"] = _build()
    nc = _CACHE["nc"]
    in_maps = _prepare_in_maps(inputs)
    res = run_bass_kernel_spmd(
        nc, in_maps, core_ids=list(range(CORES)), trace=trace
    )
    out = np.empty((B, N, D), np.float32)
    for c in range(CORES):
        b, r0 = c // GROUP, (c % GROUP) * S
        out[b, r0:r0 + S] = res.results[c]["out_rows"]
    _CACHE["last_result"] = res
    return out
